# revision 59
# baseline (speedup 1.0000x reference)
"""GCN (2x GCNConv + mean-pool + fc + LayerNorm) on 8 Trainium2 NeuronCores.

conv1 (pull): nodes+in-edges sharded by dst; x' replicated, all gathers
local. conv2 (push): edges sharded by SRC core; each core gathers its own
h1' rows (all local, no collective wait), scatter-adds per-dst partials
over the full graph, then ONE ReduceScatter delivers each core its summed
node slice — replacing the AllGather entirely.

The GCN normalization dinv[src]*dinv[dst] is factored into a host-side
source prescale (x' = x*dinv; h1' = relu(h1)*dinv on device) plus a
per-dst-node post-scale folded into the relu activation's scale operand
(exact since dinv > 0), so NO per-edge multiply is needed: each GCNConv is
  gather (gpsimd dma_gather, staged per <=64-chunk window)
  -> one merged duplicate-free scatter-add per window into u[dst] (bf16)
  -> read u cells back, add, PE transpose + matmul (z @ W) -> relu*scale.
Pooling is a PSUM-accumulated matmul with an on-device one-hot
graph-assignment matrix; pooled sums are AllReduced; the tiny fc+LayerNorm
head is computed redundantly on all cores.
"""
import sys

if '/opt/trn_rl_repo' not in sys.path:
    sys.path.insert(0, '/opt/trn_rl_repo')

import ml_dtypes
import numpy as np

import concourse.bacc as bacc
import concourse.mybir as mybir
from concourse.tile import TileContext
from concourse.bass_utils import run_bass_kernel_spmd

# ---------------------------------------------------------------- constants
N = 100000
E = 800000
IN = 64
HID = 128
G = 256
NC = 8
RPC = N // NC              # 12500 rows (nodes) per core
NCHUNK = (RPC + 127) // 128        # 98
RPC_PAD = NCHUNK * 128             # 12544
TRASH = 2048
NBLK1 = (2 * RPC_PAD + TRASH) // 256 + 1   # conv1 SBUF cell blocks (106)
TRASH2 = 256
U2BLK = RPC_PAD + TRASH2           # conv2 per-dst-range block (node | trash)
LN_EPS = 1e-5
WCHUNK = 36                # scatter window: chunks of 128 edges per window
GRP1 = 14                   # mm1 chunks per DMA group (divides 98)
GRP2 = 14                   # mm2 chunks per DMA group
F32 = mybir.dt.float32
BF16 = mybir.dt.bfloat16
I16 = mybir.dt.int16
NG = 4                     # gather groups = src % 4 (int16 index range)
DEFER_SCATTER = True       # emit scatter(w) after gather(w+1) (Pool pipelining)


def _wrap16(a, cols):
    """[n] -> [128, cols] int16: element i -> [i%16, i//16], tiled x8."""
    out = np.zeros((16, cols), np.int16)
    w = a.reshape(-1, 16).T
    out[:, : w.shape[1]] = w
    return np.tile(out, (8, 1))


def _host_prep(edge_index, batch):
    """Build per-core padded edge streams + common window/call structure."""
    src = np.asarray(edge_index[0], np.int64)
    dst = np.asarray(edge_index[1], np.int64)
    deg = (np.bincount(dst, minlength=N) + 1.0).astype(np.float32)
    dinv = (1.0 / np.sqrt(deg)).astype(np.float32)

    cores = []
    for c in range(NC):
        m = (dst >= c * RPC) & (dst < (c + 1) * RPC)
        sl = np.arange(RPC, dtype=np.int64)
        s = np.concatenate([sl + c * RPC, src[m]])
        d = np.concatenate([sl, dst[m] - c * RPC])
        # rank within dst; stable sort puts the self-edge (listed first) at 0
        order = np.argsort(d, kind='stable')
        ds = d[order]
        starts = np.r_[0, np.flatnonzero(np.diff(ds)) + 1]
        seg_len = np.diff(np.r_[starts, len(ds)])
        rk_sorted = np.arange(len(ds)) - np.repeat(starts, seg_len)
        rank = np.empty(len(ds), np.int64)
        rank[order] = rk_sorted
        cores.append((s, d, rank))

    sb_count = max(int(r.max()) for _, _, r in cores) // 2 + 1

    # common segment sizes SEG[sb][g4] (multiples of 128; max over cores);
    # g4 = src % 4 selects the 4-way-split gather table (int16 index range).
    seg = np.zeros((sb_count, NG), np.int64)
    percore_segs = []
    for ci, (s, d, rank) in enumerate(cores):
        sb = rank // 2
        g = s % 4
        cnt = np.zeros((sb_count, NG), np.int64)
        np.add.at(cnt, (sb, g), 1)
        percore_segs.append(cnt)
        seg = np.maximum(seg, cnt)
    seg = (seg + 127) // 128 * 128

    span = seg.sum(axis=1)              # edges per super-batch (mult of 128)
    sb_off = np.r_[0, np.cumsum(span)]
    epad = int(sb_off[-1])
    ecols = epad // 16

    # window/call structure (identical for every core): windows of <=WCHUNK
    # chunks within one super-batch; each window = several per-group gathers
    # into one contiguous staging tile + ONE merged duplicate-free scatter.
    calls = []   # (wlo, whi, [(g4, clo, chi)...]) in global chunk coords
    for b in range(sb_count):
        base = int(sb_off[b]) // 128
        g_edges = [int(x) // 128 for x in seg[b]]
        g_bounds = np.r_[0, np.cumsum(g_edges)]           # chunks, sb-rel
        nchunks_b = int(g_bounds[-1])
        for wlo in range(0, nchunks_b, WCHUNK):
            whi = min(wlo + WCHUNK, nchunks_b)
            pieces = []
            for g in range(NG):
                lo = max(wlo, int(g_bounds[g]))
                hi = min(whi, int(g_bounds[g + 1]))
                if lo < hi:
                    pieces.append((g, base + lo, base + hi))
            calls.append((base + wlo, base + whi, pieces))

    # per-core conv1 streams
    per_core = []
    for ci, (s, d, rank) in enumerate(cores):
        sb = rank // 2
        slot = rank % 2
        g = s % 4
        key = sb * NG + g
        order = np.argsort(key, kind='stable')
        s, d, sb, slot, g = (x[order] for x in (s, d, sb, slot, g))
        cnt = percore_segs[ci]
        gi1 = np.zeros(epad, np.int16)
        si = np.empty(epad, np.int64)
        si[:] = 2 * RPC_PAD + (np.arange(epad) % TRASH)   # default: trash
        ptr = 0
        for b in range(sb_count):
            for gg in range(NG):
                n = int(cnt[b][gg])
                off = int(sb_off[b]) + int(seg[b][:gg].sum())
                sl = slice(ptr, ptr + n)
                gi1[off:off + n] = (s[sl] // 4).astype(np.int16)
                si[off:off + n] = d[sl] + RPC_PAD * slot[sl]
                ptr += n
        per_core.append({
            "gi1": _wrap16(gi1, ecols),
            "si": _wrap16(si.astype(np.int16), ecols),
        })

    # ---- conv2 (push): edges owned by their SRC core, dst over full graph.
    # Self-loops are EXCLUDED here (they'd skew rank-0 cells by +12500 on the
    # owner's range and blow up the max-over-cores padding); the self term
    # h1'[d] is added locally from hpart during the mm2 z-assembly instead.
    cores2 = []
    for c in range(NC):
        m = (src >= c * RPC) & (src < (c + 1) * RPC)
        s2 = src[m]
        d2 = dst[m]
        order = np.argsort(d2, kind='stable')
        ds = d2[order]
        starts = np.r_[0, np.flatnonzero(np.diff(ds)) + 1]
        seg_len = np.diff(np.r_[starts, len(ds)])
        rk_sorted = np.arange(len(ds)) - np.repeat(starts, seg_len)
        rank2 = np.empty(len(ds), np.int64)
        rank2[order] = rk_sorted                  # single slot: sb2 = rank
        cores2.append((s2, d2, rank2))

    sb2_count = max(int(r.max()) for _, _, r in cores2) + 1
    seg2 = np.zeros((sb2_count, NC), np.int64)
    percore_segs2 = []
    for ci, (s2, d2, rank2) in enumerate(cores2):
        rng = d2 // RPC
        cnt = np.zeros((sb2_count, NC), np.int64)
        np.add.at(cnt, (rank2, rng), 1)
        percore_segs2.append(cnt)
        seg2 = np.maximum(seg2, cnt)
    seg2 = (seg2 + 127) // 128 * 128
    span2 = seg2.sum(axis=1)
    sb2_off = np.r_[0, np.cumsum(span2)]
    epad2 = int(sb2_off[-1])
    ecols2 = epad2 // 16

    # conv2 windows: one gather + one scatter per window; a window stays
    # inside one (sb2, dst-range) segment (scatter out_ap offset = range).
    calls2 = []   # (wlo, whi, range)
    for b in range(sb2_count):
        base = int(sb2_off[b]) // 128
        for r in range(NC):
            lo = base + int(seg2[b][:r].sum()) // 128
            hi = lo + int(seg2[b][r]) // 128
            for wlo in range(lo, hi, WCHUNK):
                calls2.append((wlo, min(wlo + WCHUNK, hi), r))

    for ci, (s2, d2, rank2) in enumerate(cores2):
        rng = d2 // RPC
        key = rank2 * NC + rng
        order = np.argsort(key, kind='stable')
        s2, d2, rank2, rng = (x[order] for x in (s2, d2, rank2, rng))
        cnt = percore_segs2[ci]
        gi2 = np.zeros(epad2, np.int16)
        si2 = np.empty(epad2, np.int64)
        si2[:] = RPC_PAD + (np.arange(epad2) % TRASH2)    # default: trash
        ptr = 0
        for b in range(sb2_count):
            for r in range(NC):
                n = int(cnt[b][r])
                off = int(sb2_off[b]) + int(seg2[b][:r].sum())
                sl = slice(ptr, ptr + n)
                gi2[off:off + n] = (s2[sl] - ci * RPC).astype(np.int16)
                si2[off:off + n] = d2[sl] - rng[sl] * RPC
                ptr += n
        per_core[ci]["gi2"] = _wrap16(gi2, ecols2)
        per_core[ci]["si2"] = _wrap16(si2.astype(np.int16), ecols2)

    # per-node graph ids (pad chunks -> -1), per-core [128, NCHUNK] f32
    gid = np.asarray(batch, np.int64)
    for ci in range(NC):
        gv = np.full(RPC_PAD, -1.0, np.float32)
        gv[:RPC] = gid[ci * RPC:(ci + 1) * RPC].astype(np.float32)
        per_core[ci]["gid"] = gv.reshape(NCHUNK, 128).T.copy()   # [128,NCHUNK]
        dv = np.zeros(RPC_PAD, np.float32)
        dv[:RPC] = dinv[ci * RPC:(ci + 1) * RPC]
        dvt = dv.reshape(NCHUNK, 128).T.copy()
        per_core[ci]["dinv1"] = dvt                               # [128,NCHUNK]
        per_core[ci]["dinv2"] = (dvt * dvt).copy()

    cntg = np.bincount(gid, minlength=G).astype(np.float32)
    inv_cnt = (1.0 / np.maximum(cntg, 1.0)).astype(np.float32)
    inv_cnt_w = inv_cnt.reshape(2, 128).T.copy()                  # [128, 2]

    meta = {"sb_count": sb_count, "epad": epad, "ecols": ecols,
            "calls": calls, "sb2_count": sb2_count, "epad2": epad2,
            "ecols2": ecols2, "calls2": calls2}
    return per_core, inv_cnt_w, dinv, meta


def _build(meta, bias_zero=True, stage=5):
    """Build + compile the 8-core Bass kernel for the given edge structure.

    stage: 1=conv1 scatter, 2=+conv1 matmul+AllGather, 4=+conv2 scatter,
    5=full. Stages <5 write debug tensors.
    """
    nc = bacc.Bacc("TRN2", target_bir_lowering=False, debug=False,
                   num_devices=NC, num_swdge_queues=1,
                   dynamic_dma_scratch_size=32768)
    epad, ecols = meta["epad"], meta["ecols"]
    calls = meta["calls"]
    ecols2, calls2 = meta["ecols2"], meta["calls2"]

    # ------------------------------------------------ I/O declarations
    xp_d = nc.dram_tensor("xp", [N, IN], F32, kind="ExternalInput")
    w1_d = nc.dram_tensor("w1", [IN, HID], F32, kind="ExternalInput")
    w2_d = nc.dram_tensor("w2", [HID, HID], F32, kind="ExternalInput")
    wfc_d = nc.dram_tensor("wfc", [HID, HID], F32, kind="ExternalInput")
    bfcr_d = nc.dram_tensor("bfcr", [128, HID], F32, kind="ExternalInput")
    gamr_d = nc.dram_tensor("gamr", [128, HID], F32, kind="ExternalInput")
    betr_d = nc.dram_tensor("betr", [128, HID], F32, kind="ExternalInput")
    if not bias_zero:
        b1r_d = nc.dram_tensor("b1r", [128, HID], F32, kind="ExternalInput")
        b2r_d = nc.dram_tensor("b2r", [128, HID], F32, kind="ExternalInput")
    gi1_d = nc.dram_tensor("gi1", [128, ecols], I16, kind="ExternalInput")
    gi2_d = nc.dram_tensor("gi2", [128, ecols2], I16, kind="ExternalInput")
    si_d = nc.dram_tensor("si", [128, ecols], I16, kind="ExternalInput")
    si2_d = nc.dram_tensor("si2", [128, ecols2], I16, kind="ExternalInput")
    gid_d = nc.dram_tensor("gid", [128, NCHUNK], F32, kind="ExternalInput")
    dinv1_d = nc.dram_tensor("dinv1", [128, NCHUNK], F32, kind="ExternalInput")
    dinv2_d = nc.dram_tensor("dinv2", [128, NCHUNK], F32, kind="ExternalInput")
    icnt_d = nc.dram_tensor("icnt", [128, 2], F32, kind="ExternalInput")
    u2_d = nc.dram_tensor("u2i", [NC * U2BLK, HID], BF16)
    y_d = nc.dram_tensor("y", [G, HID], F32, kind="ExternalOutput")
    if stage == 1:
        dbg_u = [nc.dram_tensor(f"dbg_u{p}", [128, NBLK1 * IN], BF16,
                                kind="ExternalOutput") for p in range(2)]
    if stage == 2:
        dbg_h = nc.dram_tensor("dbg_h", [RPC_PAD, HID], BF16,
                               kind="ExternalOutput")
    if stage in (3, 4):
        dbg_v = nc.dram_tensor("dbg_v", [U2BLK, HID], BF16,
                               kind="ExternalOutput")

    eye_d = nc.inline_tensor(np.eye(128, dtype=np.float32), name="eye128")
    iota_d = nc.inline_tensor(
        np.tile(np.arange(256, dtype=np.float32), (128, 1)), name="iota256")

    hpart = nc.dram_tensor("hpart", [RPC_PAD, HID], BF16)
    u2own = nc.dram_tensor("u2own", [U2BLK, HID], BF16)
    pool_loc = nc.dram_tensor("pool_loc", [G, HID], BF16)
    pool_glob = nc.dram_tensor("pool_glob", [G, HID], BF16,
                               addr_space="Shared")

    x4 = xp_d.ap().rearrange("(a b) d -> a b d", b=4)         # [25000,4,64]

    # persistent SBUF (index streams)
    gi1_s = nc.alloc_sbuf_tensor("gi1_s", [128, ecols], I16)
    gi2_s = nc.alloc_sbuf_tensor("gi2_s", [128, ecols2], I16)
    si_s = nc.alloc_sbuf_tensor("si_s", [128, ecols], I16)
    si2_s = nc.alloc_sbuf_tensor("si2_s", [128, ecols2], I16)
    # conv1 aggregation cells live in SBUF: scatter-add (parity-split CCE)
    # routes idx = d + RPC_PAD*slot to partition d%128 of buffer (d//128)%2
    # at free block (d//128)//2 + 49*slot (see dma_scatter_add SBUF mode).
    u1sb = [nc.alloc_sbuf_tensor(f"u1s{p}", [128, NBLK1 * IN], BF16)
            for p in range(2)]

    CORES = [list(range(NC))]

    def nextq():
        # single SWDGE queue: the 8 global DMA sems are handed out in
        # SCHEDULED order, and a sem may only be incremented from the queue
        # it is locked to — one queue is the only reorder-robust mapping.
        return 0

    def conv1_scatter(tc, pool):
        """staged gathers per window -> one merged scatter-add per window.
        Scatter emission trails the next window's gathers by one so the
        in-order Pool sequencer isn't parked on the scatter's data wait.
        Queue numbers are assigned at emission time (sems are queue-bound
        in emission order)."""
        pend = []
        for wi, (wlo, whi, pieces) in enumerate(calls):
            W = whi - wlo
            t = pool.tile([128, W, IN], F32, tag="gt1")
            for (g4, clo, chi) in pieces:
                n = (chi - clo) * 128
                nc.gpsimd.dma_gather(
                    t[:, clo - wlo:chi - wlo, :], x4[:, g4, :],
                    gi1_s[:, clo * 8:chi * 8],
                    n, n, IN, elem_step=4 * IN, queue_num=nextq(),
                    single_packet=False,
                )
            # f32 -> bf16 on the (nearly idle) Activation engine; the
            # scatter payload is then 128B against a 256B row stride.
            tb = pool.tile([128, W, IN], BF16, tag="gb")
            nc.vector.tensor_copy(tb[:], t[:])
            if len(pend) >= 2 and DEFER_SCATTER:
                pend.pop(0)()
            def mk(tb=tb, wlo=wlo, whi=whi):
                nc.gpsimd.dma_scatter_add(
                    u1sb[0][:], tb[:],
                    si_s[:, wlo * 8:whi * 8],
                    (whi - wlo) * 128, (whi - wlo) * 128, IN,
                    sbuf_tokens_per_rank=128, parity_reg=0,
                    out_ap_other=u1sb[1][:],
                    queue_num=nextq(), single_packet=False,
                )
            if DEFER_SCATTER:
                pend.append(mk)
            else:
                mk()
        while pend:
            pend.pop(0)()

    def conv2_scatter(tc, pool):
        """push conv2: gather own h1' rows, scatter per-dst-range partials."""
        hp = hpart.ap()
        pend = []
        for wi, (wlo, whi, r) in enumerate(calls2):
            W = whi - wlo
            t = pool.tile([128, W, HID], BF16, tag="gt2")
            n = W * 128
            nc.gpsimd.dma_gather(
                t[:], hp, gi2_s[:, wlo * 8:whi * 8],
                n, n, HID, queue_num=nextq(), single_packet=False,
            )
            if pending is not None and DEFER_SCATTER and wi >= 2:
                pending()
                pending = None
            def mk(t=t, wlo=wlo, whi=whi, r=r):
                nc.gpsimd.dma_scatter_add(
                    u2_d.ap()[r * U2BLK:(r + 1) * U2BLK, :], t[:],
                    si2_s[:, wlo * 8:whi * 8],
                    (whi - wlo) * 128, (whi - wlo) * 128, HID,
                    queue_num=nextq(), single_packet=False,
                )
            if DEFER_SCATTER:
                pend.append(mk)
            else:
                mk()
        while pend:
            pend.pop(0)()

    with TileContext(nc) as tc:
        with tc.tile_pool(name="init", bufs=1) as ipool:
            nc.sync.dma_start(out=gi1_s[:], in_=gi1_d[:])
            nc.sync.dma_start(out=gi2_s[:], in_=gi2_d[:])
            nc.sync.dma_start(out=si_s[:], in_=si_d[:])
            nc.sync.dma_start(out=si2_s[:], in_=si2_d[:])
            eye_t = ipool.tile([128, 128], F32)
            nc.sync.dma_start(out=eye_t[:], in_=eye_d[:])
            w1_t = ipool.tile([IN, HID], F32)
            nc.sync.dma_start(out=w1_t[:], in_=w1_d[:])
            w2_t = ipool.tile([HID, HID], F32)
            nc.sync.dma_start(out=w2_t[:], in_=w2_d[:])
            gid_t = ipool.tile([128, NCHUNK], F32)
            nc.sync.dma_start(out=gid_t[:], in_=gid_d[:])
            dinv1_t = ipool.tile([128, NCHUNK], F32)
            nc.sync.dma_start(out=dinv1_t[:], in_=dinv1_d[:])
            dinv2_t = ipool.tile([128, NCHUNK], F32)
            nc.sync.dma_start(out=dinv2_t[:], in_=dinv2_d[:])
            iota_t = ipool.tile([128, 256], F32)
            nc.sync.dma_start(out=iota_t[:], in_=iota_d[:])
            if not bias_zero:
                b1r_t = ipool.tile([128, HID], F32)
                nc.sync.dma_start(out=b1r_t[:], in_=b1r_d[:])
                b2r_t = ipool.tile([128, HID], F32)
                nc.sync.dma_start(out=b2r_t[:], in_=b2r_d[:])

            # ---------------- conv1: edge aggregation into SBUF cells
            nc.vector.memset(u1sb[0][:], 0.0)
            nc.vector.memset(u1sb[1][:], 0.0)
            with tc.tile_pool(name="sc1", bufs=4) as spool:
                conv1_scatter(tc, spool)
            # zero-fill the internal conv2 partial buffer (the collective
            # may not read IO tensors, so it cannot be host-zeroed). Emitted
            # after conv1's scatters so the SP-queue copies land in the mm1
            # DMA lull; only conv2's scatters depend on them.
            nc.vector.memset(u2_d.ap()[:, :], 0.0)
            if stage == 1:
                for p in range(2):
                    nc.sync.dma_start(out=dbg_u[p][:], in_=u1sb[p][:])

            # -------- conv1: h1' = relu(z @ W1 * dinv^2)  (prescaled for
            # conv2; exact since relu(a*d)*d == relu(a*d^2) for d>0)
            hp_r = hpart.ap().rearrange("(a p) f -> p a f", p=128)
            u1v = [u.ap().rearrange("p (g f) -> p g f", f=IN) for u in u1sb]
            with (
                tc.tile_pool(name="mm1", bufs=4) as mpool,
                tc.tile_pool(name="ps1", bufs=4, space="PSUM") as ppool,
            ):
                for a0 in range(0, NCHUNK, GRP1):
                    z = mpool.tile([128, GRP1, IN], F32, tag="z")
                    for j in range(GRP1):
                        a = a0 + j
                        buf = u1v[a % 2]
                        g0 = a // 2
                        eng = nc.gpsimd
                        eng.tensor_add(z[:, j, :], buf[:, g0, :],
                                       buf[:, g0 + 49, :])
                    hg = mpool.tile([128, GRP1, HID], BF16, tag="hg")
                    for j in range(GRP1):
                        a = a0 + j
                        zT_p = ppool.tile([IN, 128], F32, tag="zT")
                        nc.tensor.transpose(zT_p[:], z[:, j, :], eye_t[:])
                        zT_s = mpool.tile([IN, 128], F32, tag="zTs")
                        nc.scalar.activation(
                            zT_s[:], zT_p[:],
                            mybir.ActivationFunctionType.Copy)
                        h_p = ppool.tile([128, HID], F32, tag="hp")
                        nc.tensor.matmul(h_p[:], zT_s[:], w1_t[:])
                        if bias_zero:
                            nc.scalar.activation(
                                hg[:, j, :], h_p[:],
                                mybir.ActivationFunctionType.Relu,
                                scale=dinv2_t[:, a:a + 1])
                        else:
                            nc.vector.tensor_scalar(
                                h_p[:], h_p[:], dinv1_t[:, a:a + 1], None,
                                mybir.AluOpType.mult)
                            nc.vector.tensor_add(h_p[:], h_p[:], b1r_t[:])
                            hr = mpool.tile([128, HID], F32, tag="hr")
                            nc.scalar.activation(
                                hr[:], h_p[:],
                                mybir.ActivationFunctionType.Relu)
                            nc.vector.tensor_scalar(
                                hg[:, j, :], hr[:], dinv1_t[:, a:a + 1], None,
                                mybir.AluOpType.mult)
                    nc.sync.dma_start(out=hp_r[:, a0:a0 + GRP1], in_=hg[:])

            # ---------------- conv2: push partials + ReduceScatter
            if stage == 2:
                nc.sync.dma_start(out=dbg_h[:], in_=hpart[:])
            # keep a whole-hpart SBUF copy for the mm2 self-term; loads
            # during the conv2 gather/scatter phase, off the mm2 tail.
            hpsb = ipool.tile([128, NCHUNK, HID], BF16, name="hpsb")
            nc.sync.dma_start(
                out=hpsb[:],
                in_=hpart.ap().rearrange("(a p) f -> p a f", p=128))
            if stage >= 4:
                with tc.tile_pool(name="sc2", bufs=4) as spool:
                    conv2_scatter(tc, spool)
                nc.gpsimd.collective_compute(
                    "ReduceScatter", mybir.AluOpType.add, CORES,
                    [u2_d[:]], [u2own[:]],
                )
            if stage == 4:
                nc.sync.dma_start(out=dbg_v[:], in_=u2own[:])

            if stage >= 5:
                # ---------------- conv2 matmul + relu + pooling matmul
                u2or = u2own.ap()[:NCHUNK * 128, :] \
                    .rearrange("(a p) f -> p a f", p=128)
                with (
                    tc.tile_pool(name="mm2", bufs=6) as mpool,
                    tc.tile_pool(name="ps2", bufs=4, space="PSUM") as ppool,
                    tc.tile_pool(name="pacc", bufs=1, space="PSUM") as accpool,
                ):
                    pooled = [accpool.tile([128, HID], F32, tag=f"pool{h}",
                                           name=f"pooled{h}")
                              for h in range(4)]
                    for a0 in range(0, NCHUNK, GRP2):
                        ga = mpool.tile([128, GRP2, HID], BF16, tag="ua")
                        nc.sync.dma_start(out=ga[:], in_=u2or[:, a0:a0 + GRP2])
                        z = mpool.tile([128, GRP2, HID], F32, tag="z")
                        nc.vector.tensor_add(z[:], ga[:],
                                             hpsb[:, a0:a0 + GRP2, :])
                        for j in range(GRP2):
                            a = a0 + j
                            zT_p = ppool.tile([HID, 128], F32, tag="zT")
                            nc.tensor.transpose(zT_p[:], z[:, j, :], eye_t[:])
                            zT_s = mpool.tile([HID, 128], F32, tag="zTs")
                            nc.scalar.activation(
                                zT_s[:], zT_p[:],
                                mybir.ActivationFunctionType.Copy)
                            h_p = ppool.tile([128, HID], F32, tag="hp")
                            nc.tensor.matmul(h_p[:], zT_s[:], w2_t[:])
                            h2_s = mpool.tile([128, HID], F32, tag="h2s")
                            if bias_zero:
                                nc.scalar.activation(
                                    h2_s[:], h_p[:],
                                    mybir.ActivationFunctionType.Relu,
                                    scale=dinv1_t[:, a:a + 1])
                            else:
                                nc.vector.tensor_scalar(
                                    h_p[:], h_p[:], dinv1_t[:, a:a + 1], None,
                                    mybir.AluOpType.mult)
                                nc.vector.tensor_add(h_p[:], h_p[:], b2r_t[:])
                                nc.scalar.activation(
                                    h2_s[:], h_p[:],
                                    mybir.ActivationFunctionType.Relu)
                            sel = mpool.tile([128, 256], F32, tag="sel")
                            nc.vector.tensor_tensor(
                                sel[:],
                                gid_t[:, a:a + 1].broadcast_to([128, 256]),
                                iota_t[:],
                                mybir.AluOpType.is_equal,
                            )
                            for hh in range(2):
                                nc.tensor.matmul(
                                    pooled[hh + 2 * (a % 2)][:],
                                    sel[:, hh * 128:(hh + 1) * 128], h2_s[:],
                                    start=(a < 2), stop=(a >= NCHUNK - 2),
                                )
                    # pooled sums -> dram
                    pl_r = pool_loc.ap().rearrange("(h p) f -> p h f", p=128)
                    pl_s = mpool.tile([128, 2, HID], BF16, tag="pls")
                    nc.vector.tensor_add(pl_s[:, 0, :], pooled[0][:],
                                         pooled[2][:])
                    nc.vector.tensor_add(pl_s[:, 1, :], pooled[1][:],
                                         pooled[3][:])
                    nc.sync.dma_start(out=pl_r[:], in_=pl_s[:])

                nc.gpsimd.collective_compute(
                    "AllReduce", mybir.AluOpType.add, CORES,
                    [pool_loc[:]], [pool_glob[:]],
                )

                # ---------------- head: mean-div, fc, LayerNorm (tiny)
                pg_r = pool_glob.ap().rearrange("(h p) f -> p h f", p=128)
                y_r = y_d.ap().rearrange("(h p) f -> p h f", p=128)
                with (
                    tc.tile_pool(name="head", bufs=1) as hpool,
                    tc.tile_pool(name="psh", bufs=2, space="PSUM") as hps,
                ):
                    wfc_t = hpool.tile([HID, HID], F32)
                    nc.sync.dma_start(out=wfc_t[:], in_=wfc_d[:])
                    bfcr_t = hpool.tile([128, HID], F32)
                    nc.sync.dma_start(out=bfcr_t[:], in_=bfcr_d[:])
                    gamr_t = hpool.tile([128, HID], F32)
                    nc.sync.dma_start(out=gamr_t[:], in_=gamr_d[:])
                    betr_t = hpool.tile([128, HID], F32)
                    nc.sync.dma_start(out=betr_t[:], in_=betr_d[:])
                    icnt_t = hpool.tile([128, 2], F32)
                    nc.sync.dma_start(out=icnt_t[:], in_=icnt_d[:])
                    eps_t = hpool.tile([128, 1], F32)
                    nc.vector.memset(eps_t[:], LN_EPS)
                    yo = hpool.tile([128, 2, HID], F32)
                    for hh in range(2):
                        pgb = hpool.tile([128, HID], BF16, tag="pgb")
                        nc.sync.dma_start(out=pgb[:], in_=pg_r[:, hh, :])
                        pg_s = hpool.tile([128, HID], F32, tag="pg")
                        nc.vector.tensor_scalar(
                            pg_s[:], pgb[:], icnt_t[:, hh:hh + 1], None,
                            mybir.AluOpType.mult)
                        pgT_p = hps.tile([HID, 128], F32, tag="pgT")
                        nc.tensor.transpose(pgT_p[:], pg_s[:], eye_t[:])
                        pgT_s = hpool.tile([HID, 128], F32, tag="pgTs")
                        nc.vector.tensor_copy(pgT_s[:], pgT_p[:])
                        y_p = hps.tile([128, HID], F32, tag="yp")
                        nc.tensor.matmul(y_p[:], pgT_s[:], wfc_t[:])
                        y_s = hpool.tile([128, HID], F32, tag="ys")
                        nc.vector.tensor_add(y_s[:], y_p[:], bfcr_t[:])
                        # LayerNorm along features (free dim)
                        mu = hpool.tile([128, 1], F32, tag="mu")
                        nc.vector.tensor_reduce(mu[:], y_s[:],
                                                mybir.AxisListType.XYZW,
                                                mybir.AluOpType.add)
                        nc.vector.tensor_scalar(mu[:], mu[:], -1.0 / HID, None,
                                                mybir.AluOpType.mult)
                        cen = hpool.tile([128, HID], F32, tag="cen")
                        nc.vector.tensor_scalar(cen[:], y_s[:], mu[:], None,
                                                mybir.AluOpType.add)
                        sq = hpool.tile([128, HID], F32, tag="sq")
                        nc.vector.tensor_mul(sq[:], cen[:], cen[:])
                        var = hpool.tile([128, 1], F32, tag="var")
                        nc.vector.tensor_reduce(var[:], sq[:],
                                                mybir.AxisListType.XYZW,
                                                mybir.AluOpType.add)
                        std = hpool.tile([128, 1], F32, tag="std")
                        nc.scalar.activation(std[:], var[:],
                                             mybir.ActivationFunctionType.Sqrt,
                                             bias=eps_t[:], scale=1.0 / HID)
                        rstd = hpool.tile([128, 1], F32, tag="rstd")
                        nc.vector.reciprocal(rstd[:], std[:])
                        nc.vector.tensor_scalar(cen[:], cen[:], rstd[:], None,
                                                mybir.AluOpType.mult)
                        nc.vector.tensor_mul(cen[:], cen[:], gamr_t[:])
                        nc.vector.tensor_add(yo[:, hh, :], cen[:], betr_t[:])
                    nc.sync.dma_start(out=y_r[:], in_=yo[:])

    nc.compile()
    return nc


_CACHE = {}


def make_in_maps(x, edge_index, batch, W1, b1, W2, b2, Wfc, bfc, gamma, beta,
                 per_core=None, inv_cnt_w=None, dinv=None, meta=None):
    if per_core is None:
        per_core, inv_cnt_w, dinv, meta = _host_prep(
            np.asarray(edge_index), np.asarray(batch))
    x = np.asarray(x, np.float32)
    xp = x * dinv[:, None]
    rep = lambda v: np.tile(np.asarray(v, np.float32)[None, :], (128, 1))
    bias_zero = (not np.any(np.asarray(b1))) and (not np.any(np.asarray(b2)))
    shared = {
        "xp": xp,
        "w1": np.asarray(W1, np.float32),
        "w2": np.asarray(W2, np.float32),
        "wfc": np.asarray(Wfc, np.float32),
        "bfcr": rep(bfc),
        "gamr": rep(gamma), "betr": rep(beta),
        "icnt": inv_cnt_w,
    }
    if not bias_zero:
        shared["b1r"] = rep(b1)
        shared["b2r"] = rep(b2)
    in_maps = []
    for c in range(NC):
        m = dict(shared)
        for k in ("gi1", "gi2", "si", "si2", "gid", "dinv1", "dinv2"):
            m[k] = per_core[c][k]
        in_maps.append(m)
    return in_maps, bias_zero, meta


def kernel(x, edge_index, batch, W1, b1, W2, b2, Wfc, bfc, gamma, beta,
           _stage=5, _full_results=False):
    per_core, inv_cnt_w, dinv, meta = _host_prep(np.asarray(edge_index),
                                                 np.asarray(batch))
    in_maps, bias_zero, meta = make_in_maps(
        x, edge_index, batch, W1, b1, W2, b2, Wfc, bfc, gamma, beta,
        per_core, inv_cnt_w, dinv, meta)
    key = (meta["epad"], meta["sb_count"], len(meta["calls"]), bias_zero,
           _stage)
    if key not in _CACHE:
        _CACHE[key] = _build(meta, bias_zero, _stage)
    nc = _CACHE[key]

    res = run_bass_kernel_spmd(nc, in_maps, list(range(NC)))
    if _full_results:
        return res.results
    return res.results[0]["y"]
